# revision 1
# baseline (speedup 1.0000x reference)
"""Canny edge detector on 8 Trainium2 NeuronCores (Bass/Tile).

Self-contained: accepts the FULL img [32,3,512,512] f32, shards the batch
across 8 cores (4 images each), runs one SPMD Bass program, and returns the
full 6-tuple of outputs matching reference.py.

Algorithm (per core, per image):
  All separable convolutions run on the TensorEngine as matmuls in a
  "data-stationary / moving-band" form: lhsT = a 128x128 image block,
  rhs = a banded Toeplitz window [128, 256] for the 1-D kernel. Each such
  matmul computes a same-padded 1-D conv along the partition axis of the
  data AND transposes the layout, so alternating phases (y, x, y, x) return
  to the natural layout with zero explicit transposes:
    P1: B1^T = (Gv5 * X)^T      P2: BL = Gh5 * B1   (-> blurred output)
    P3: A2^T = ([1,2,1]y * BL)^T, A3^T = ([1,0,-1]y * BL)^T
    P4: 3*gx = sum_c [1,0,-1]x * A2_c, 3*gy = sum_c [1,2,1]x * A3_c
  (channel mean folded into P4 PSUM accumulation; the 1/3 is applied via
  the sqrt scale since every decision in the pipeline is scale-invariant)
  Pointwise magnitude/orientation/NMS/thresholds run on DVE + ACT with
  uint8 predicate masks + copy_predicated selection; vertical NMS
  neighbours come from row-shifted SBUF->SBUF DMA copies of mag^2.
"""
import sys
import numpy as np

sys.path.insert(0, "/opt/trn_rl_repo")

import concourse.bacc as bacc
import concourse.mybir as mybir
from concourse.tile import TileContext

F32 = mybir.dt.float32
F32R = mybir.dt.float32r
BF16 = mybir.dt.bfloat16
U8 = mybir.dt.uint8
AL = mybir.AluOpType
ACTF = mybir.ActivationFunctionType

H = W = 512
NT = 4
WIN = 256
WSTARTS = (0, 64, 192, 256)
T1SQ = float(np.tan(np.deg2rad(22.5)) ** 2)
T2SQ = float(np.tan(np.deg2rad(67.5)) ** 2)
THR = 0.2
PADW = 516
N_CORES = 8
B_PER_CORE = 4
CONV_DT = F32   # exact fp32 matmuls (4 cyc/row) — fp32r is 4x faster but too noisy


def _gauss5():
    n = np.arange(5) - 2.0
    g = np.exp(-0.5 * n ** 2)
    return (g / g.sum()).astype(np.float32).astype(np.float64)


def _make_band(kern, scale=1.0):
    L = len(kern)
    c = (L - 1) // 2
    out = np.zeros((128, NT * WIN), dtype=np.float32)
    for t in range(NT):
        s = WSTARTS[t]
        for k in range(128):
            h = 128 * t + k
            for a in range(L):
                i = h - a + c
                n = i - s
                if 0 <= n < WIN and 0 <= i < H:
                    out[k, WIN * t + n] = np.float32(kern[a] * scale)
    return out


def band_inputs():
    g = _gauss5()
    return {
        "band_p1": _make_band(g),
        "band_p2": _make_band(g),
        "band_p3a": _make_band([1., 2., 1.]),
        "band_p3b": _make_band([1., 0., -1.]),
        "band_p4a": _make_band([1., 0., -1.]),
        "band_p4b": _make_band([1., 2., 1.]),
    }


def build(B=B_PER_CORE, conv_dt=CONV_DT, reps=1):
    nc = bacc.Bacc()
    img = nc.dram_tensor("img", [B, 3, H, W], conv_dt, kind="ExternalInput")
    bands = {}
    for name in ("band_p1", "band_p2", "band_p3a", "band_p3b", "band_p4a", "band_p4b"):
        bands[name] = nc.dram_tensor(name, [128, NT * WIN], conv_dt, kind="ExternalInput")
    zrow = nc.dram_tensor("zrow", [1, PADW], F32, kind="ExternalInput")

    o_blur = nc.dram_tensor("o_blur", [B, 3, H, W], conv_dt, kind="ExternalOutput")
    o_mag = nc.dram_tensor("o_mag", [B, 1, H, W], F32, kind="ExternalOutput")
    o_ori = nc.dram_tensor("o_ori", [B, 1, H, W], F32, kind="ExternalOutput")
    o_tmag = nc.dram_tensor("o_tmag", [B, 1, H, W], F32, kind="ExternalOutput")
    o_thin = nc.dram_tensor("o_thin", [B, 1, H, W], F32, kind="ExternalOutput")
    o_tthin = nc.dram_tensor("o_tthin", [B, 1, H, W], F32, kind="ExternalOutput")

    with TileContext(nc) as tc:
        with (
            tc.tile_pool(name="bandp", bufs=1) as bandp,
            tc.tile_pool(name="xrow", bufs=6) as xrowp,
            tc.tile_pool(name="interm", bufs=6) as intermp,
            tc.tile_pool(name="blctx", bufs=6) as blp,
            tc.tile_pool(name="a23", bufs=12) as a23p,
            tc.tile_pool(name="nms", bufs=1) as nmsp,
            tc.tile_pool(name="pw", bufs=10) as pwp,
            tc.tile_pool(name="pwb", bufs=10) as pwbp,
            tc.tile_pool(name="pwu", bufs=14) as pwup,
            tc.tile_pool(name="stg", bufs=2) as stgp,
            tc.tile_pool(name="ps12", bufs=2, space="PSUM") as psp12,
            tc.tile_pool(name="ps3", bufs=2, space="PSUM") as psp3,
            tc.tile_pool(name="ps4", bufs=1, space="PSUM") as psp4,
        ):
            band_t = {}
            for name, dram in bands.items():
                bt = bandp.tile([128, NT * WIN], conv_dt, tag=name)
                nc.sync.dma_start(bt[:], dram[:])
                band_t[name] = bt

            def bslice(name, t):
                return band_t[name][:, WIN * t: WIN * (t + 1)]

            def body(_iv=None):
                for b in range(B):
                    a2t = [None] * (3 * NT)
                    a3t = [None] * (3 * NT)
                    for c in range(3):
                        xrows = []
                        for t in range(NT):
                            xt = xrowp.tile([128, W], conv_dt, tag="xrow")
                            nc.sync.dma_start(xt[:], img[b, c, 128 * t:128 * (t + 1), :])
                            xrows.append(xt)
                        b1t = []
                        for j in range(NT):
                            ps = psp12.tile([128, W], F32, tag="p12")
                            for t in range(NT):
                                s = WSTARTS[t]
                                nc.tensor.matmul(
                                    ps[:, s:s + WIN],
                                    xrows[t][:, 128 * j:128 * (j + 1)],
                                    bslice("band_p1", t),
                                    start=(t == 0), stop=(t == NT - 1),
                                )
                            sb = intermp.tile([128, W], conv_dt, tag="b1t")
                            (nc.scalar.activation(sb[:], ps[:], ACTF.Copy) if j % 2 == 0
                             else nc.vector.tensor_copy(sb[:], ps[:]))
                            b1t.append(sb)
                        blt = []
                        for i in range(NT):
                            ps = psp12.tile([128, W], F32, tag="p12")
                            for u in range(NT):
                                s = WSTARTS[u]
                                nc.tensor.matmul(
                                    ps[:, s:s + WIN],
                                    b1t[u][:, 128 * i:128 * (i + 1)],
                                    bslice("band_p2", u),
                                    start=(u == 0), stop=(u == NT - 1),
                                )
                            sb = blp.tile([128, W], conv_dt, tag="bl")
                            (nc.scalar.activation(sb[:], ps[:], ACTF.Copy) if i % 2 == 0
                             else nc.vector.tensor_copy(sb[:], ps[:]))
                            nc.sync.dma_start(o_blur[b, c, 128 * i:128 * (i + 1), :], sb[:])
                            blt.append(sb)
                        for j in range(NT):
                            ps2 = psp3.tile([128, W], F32, tag="p3a")
                            ps3_ = psp3.tile([128, W], F32, tag="p3b")
                            for t in range(NT):
                                s = WSTARTS[t]
                                nc.tensor.matmul(
                                    ps2[:, s:s + WIN],
                                    blt[t][:, 128 * j:128 * (j + 1)],
                                    bslice("band_p3a", t),
                                    start=(t == 0), stop=(t == NT - 1),
                                )
                                nc.tensor.matmul(
                                    ps3_[:, s:s + WIN],
                                    blt[t][:, 128 * j:128 * (j + 1)],
                                    bslice("band_p3b", t),
                                    start=(t == 0), stop=(t == NT - 1),
                                )
                            sb2 = a23p.tile([128, W], conv_dt, tag="a2t")
                            sb3 = a23p.tile([128, W], conv_dt, tag="a3t")
                            (nc.scalar.activation(sb2[:], ps2[:], ACTF.Copy) if j % 2 == 0
                             else nc.vector.tensor_copy(sb2[:], ps2[:]))
                            (nc.scalar.activation(sb3[:], ps3_[:], ACTF.Copy) if j % 2 == 1
                             else nc.vector.tensor_copy(sb3[:], ps3_[:]))
                            a2t[c * NT + j] = sb2
                            a3t[c * NT + j] = sb3

                    m2p = nmsp.tile([128, NT * PADW], F32, tag="m2p")
                    magt = nmsp.tile([128, NT * W], F32, tag="magt")
                    for t in range(NT):
                        nc.gpsimd.memset(m2p[:, t * PADW:t * PADW + 2], 0.0)
                        nc.gpsimd.memset(m2p[:, t * PADW + 2 + W:(t + 1) * PADW], 0.0)

                    ori_stage = []
                    for i in range(NT):
                        psx = psp4.tile([128, W], F32, tag="p4x")
                        psy = psp4.tile([128, W], F32, tag="p4y")
                        for c in range(3):
                            for u in range(NT):
                                s = WSTARTS[u]
                                first = (c == 0 and u == 0)
                                last = (c == 2 and u == NT - 1)
                                nc.tensor.matmul(
                                    psx[:, s:s + WIN],
                                    a2t[c * NT + u][:, 128 * i:128 * (i + 1)],
                                    bslice("band_p4a", u),
                                    start=first, stop=last,
                                )
                                nc.tensor.matmul(
                                    psy[:, s:s + WIN],
                                    a3t[c * NT + u][:, 128 * i:128 * (i + 1)],
                                    bslice("band_p4b", u),
                                    start=first, stop=last,
                                )
                        # pointwise stage A (gx,gy are 3x the true values; all
                        # decisions are scale-invariant, mag scale fixed in sqrt)
                        gx2 = pwp.tile([128, W], F32, tag="pwf")
                        gy2 = pwp.tile([128, W], F32, tag="pwf")
                        nc.scalar.activation(gx2[:], psx[:], ACTF.Square)
                        nc.scalar.activation(gy2[:], psy[:], ACTF.Square)
                        sgx = pwbp.tile([128, W], BF16, tag="pwb")
                        sgy = pwbp.tile([128, W], BF16, tag="pwb")
                        nc.scalar.activation(sgx[:], psx[:], ACTF.Sign)
                        nc.scalar.activation(sgy[:], psy[:], ACTF.Sign)
                        m2c = m2p[:, i * PADW + 2: i * PADW + 2 + W]
                        nc.vector.tensor_tensor(m2c, gx2[:], gy2[:], AL.add)
                        magc = magt[:, i * W:(i + 1) * W]
                        nc.scalar.activation(magc, m2c, ACTF.Sqrt, scale=1.0 / 9.0)
                        nc.sync.dma_start(o_mag[b, 0, 128 * i:128 * (i + 1), :], magc)
                        tmg = stgp.tile([128, W], F32, tag="tmg")
                        nc.vector.scalar_tensor_tensor(tmg[:], magc, THR, magc, AL.is_ge, AL.mult)
                        nc.sync.dma_start(o_tmag[b, 0, 128 * i:128 * (i + 1), :], tmg[:])
                        t1x = pwp.tile([128, W], F32, tag="pwf")
                        nc.vector.tensor_scalar(t1x[:], gx2[:], T1SQ, None, AL.mult)
                        t2x = pwp.tile([128, W], F32, tag="pwf")
                        nc.vector.tensor_scalar(t2x[:], gx2[:], T2SQ, None, AL.mult)
                        c1u = pwup.tile([128, W], U8, tag="pwu")
                        nc.vector.tensor_tensor(c1u[:], gy2[:], t1x[:], AL.is_le)
                        c2u = pwup.tile([128, W], U8, tag="pwu")
                        nc.vector.tensor_tensor(c2u[:], gy2[:], t2x[:], AL.is_ge)
                        c1b = pwbp.tile([128, W], BF16, tag="pwb")
                        nc.vector.tensor_copy(c1b[:], c1u[:])
                        c2b = pwbp.tile([128, W], BF16, tag="pwb")
                        nc.vector.tensor_copy(c2b[:], c2u[:])
                        qq = pwbp.tile([128, W], BF16, tag="pwb")
                        nc.vector.tensor_tensor(qq[:], sgx[:], sgy[:], AL.mult)
                        qpos = pwup.tile([128, W], U8, tag="pwu")
                        nc.vector.tensor_scalar(qpos[:], qq[:], 0.0, None, AL.is_gt)
                        rr = pwbp.tile([128, W], BF16, tag="pwb")
                        nc.vector.scalar_tensor_tensor(rr[:], c2b[:], -1.0, c1b[:], AL.add, AL.subtract)
                        ss = pwbp.tile([128, W], BF16, tag="pwb")
                        nc.vector.tensor_tensor(ss[:], qq[:], rr[:], AL.mult)
                        s45 = pwp.tile([128, W], F32, tag="pwf")
                        nc.vector.tensor_scalar(s45[:], ss[:], 45.0, 180.0, AL.mult, AL.add)
                        ori = stgp.tile([128, W], F32, tag="ori")
                        nc.vector.scalar_tensor_tensor(ori[:], sgy[:], 90.0, s45[:], AL.mult, AL.add)
                        nc.sync.dma_start(o_ori[b, 0, 128 * i:128 * (i + 1), :], ori[:])
                        ori_stage.append((c1u, c2u, qpos))

                    rp = nmsp.tile([128, NT * PADW], F32, tag="rp")
                    rm = nmsp.tile([128, NT * PADW], F32, tag="rm")
                    for t in range(NT):
                        nc.sync.dma_start(rp[0:127, t * PADW:(t + 1) * PADW],
                                          m2p[1:128, t * PADW:(t + 1) * PADW])
                        if t < NT - 1:
                            nc.sync.dma_start(rp[127:128, t * PADW:(t + 1) * PADW],
                                              m2p[0:1, (t + 1) * PADW:(t + 2) * PADW])
                        else:
                            nc.sync.dma_start(rp[127:128, t * PADW:(t + 1) * PADW], zrow[0:1, :])
                        nc.sync.dma_start(rm[1:128, t * PADW:(t + 1) * PADW],
                                          m2p[0:127, t * PADW:(t + 1) * PADW])
                        if t > 0:
                            nc.sync.dma_start(rm[0:1, t * PADW:(t + 1) * PADW],
                                              m2p[127:128, (t - 1) * PADW:t * PADW])
                        else:
                            nc.gpsimd.memset(rm[0:1, t * PADW:(t + 1) * PADW], 0.0)

                    for i in range(NT):
                        c1u, c2u, qpos = ori_stage[i]
                        c0 = i * PADW + 2
                        m2c = m2p[:, c0:c0 + W]
                        sel = pwp.tile([128, W], F32, tag="pwf")
                        m0t = pwp.tile([128, W], F32, tag="pwf")
                        m1t = pwp.tile([128, W], F32, tag="pwf")
                        m2t = pwp.tile([128, W], F32, tag="pwf")
                        nc.vector.tensor_tensor(sel[:], rp[:, c0 - 1:c0 - 1 + W], rm[:, c0 + 1:c0 + 1 + W], AL.max)
                        nc.vector.tensor_tensor(m1t[:], rp[:, c0 + 1:c0 + 1 + W], rm[:, c0 - 1:c0 - 1 + W], AL.max)
                        nc.vector.tensor_tensor(m2t[:], rp[:, c0:c0 + W], rm[:, c0:c0 + W], AL.max)
                        nc.vector.tensor_tensor(m0t[:], m2p[:, c0 - 1:c0 - 1 + W], m2p[:, c0 + 1:c0 + 1 + W], AL.max)
                        nc.vector.copy_predicated(sel[:], qpos[:], m1t[:])
                        nc.vector.copy_predicated(sel[:], c2u[:], m2t[:])
                        nc.vector.copy_predicated(sel[:], c1u[:], m0t[:])
                        tmask = pwbp.tile([128, W], BF16, tag="pwb")
                        nc.vector.tensor_tensor(tmask[:], m2c, sel[:], AL.is_gt)
                        magc = magt[:, i * W:(i + 1) * W]
                        thin = stgp.tile([128, W], F32, tag="thin")
                        nc.vector.tensor_tensor(thin[:], tmask[:], magc, AL.mult)
                        nc.sync.dma_start(o_thin[b, 0, 128 * i:128 * (i + 1), :], thin[:])
                        tthin = stgp.tile([128, W], F32, tag="tthin")
                        nc.vector.scalar_tensor_tensor(tthin[:], thin[:], THR, thin[:], AL.is_ge, AL.mult)
                        nc.sync.dma_start(o_tthin[b, 0, 128 * i:128 * (i + 1), :], tthin[:])

            if reps == 1:
                body()
            else:
                tc.For_i_unrolled(0, reps, 1, body, max_unroll=1)
    nc.compile()
    return nc


# ---------------------------------------------------------------------------
# PJRT runner (axon): one SPMD program over 8 cores via shard_map
# ---------------------------------------------------------------------------
_CACHE = {}


def _get_runner():
    if "runner" in _CACHE:
        return _CACHE["runner"]
    import jax
    import concourse.bass2jax as b2j
    from jax.sharding import Mesh, PartitionSpec
    from jax.experimental.shard_map import shard_map

    nc = build()
    b2j.install_neuronx_cc_hook()
    in_names, out_names, out_avals = [], [], []
    partition_name = nc.partition_id_tensor.name if nc.partition_id_tensor else None
    for alloc in nc.m.functions[0].allocations:
        if not isinstance(alloc, mybir.MemoryLocationSet):
            continue
        name = alloc.memorylocations[0].name
        if alloc.kind == "ExternalInput":
            if name != partition_name:
                in_names.append(name)
        elif alloc.kind == "ExternalOutput":
            out_names.append(name)
            out_avals.append(jax.core.ShapedArray(tuple(alloc.tensor_shape),
                                                  mybir.dt.np(alloc.dtype)))
    n_params = len(in_names)
    all_in_names = list(in_names) + list(out_names)
    if partition_name is not None:
        all_in_names.append(partition_name)

    def _body(*args):
        operands = list(args)
        if partition_name is not None:
            operands.append(b2j.partition_id_tensor())
        outs = b2j._bass_exec_p.bind(
            *operands,
            out_avals=tuple(out_avals),
            in_names=tuple(all_in_names),
            out_names=tuple(out_names),
            lowering_input_output_aliases=(),
            sim_require_finite=False,
            sim_require_nnan=False,
            nc=nc,
        )
        return tuple(outs)

    devices = jax.devices()[:N_CORES]
    mesh = Mesh(np.asarray(devices), ("core",))
    in_specs = (PartitionSpec("core"),) * (n_params + len(out_names))
    out_specs = (PartitionSpec("core"),) * len(out_names)
    jf = jax.jit(shard_map(_body, mesh=mesh, in_specs=in_specs,
                           out_specs=out_specs, check_rep=False),
                 keep_unused=True)
    _CACHE["runner"] = (jf, in_names, out_names, out_avals)
    return _CACHE["runner"]


def kernel(img):
    img = np.ascontiguousarray(np.asarray(img, dtype=np.float32))
    assert img.shape == (N_CORES * B_PER_CORE, 3, H, W), img.shape
    jf, in_names, out_names, out_avals = _get_runner()

    shared = band_inputs()
    shared["zrow"] = np.zeros((1, PADW), np.float32)
    per_core_in = {"img": img}  # sharded along axis 0 by shard_map

    args = []
    for n in in_names:
        if n == "img":
            args.append(img)
        else:
            # replicate shared constants: concat one copy per core on axis 0
            v = shared[n]
            args.append(np.concatenate([v] * N_CORES, axis=0))
    for a in out_avals:
        args.append(np.zeros((a.shape[0] * N_CORES,) + a.shape[1:], a.dtype))

    outs = jf(*args)
    res = {n: np.asarray(o) for n, o in zip(out_names, outs)}
    return (res["o_blur"].astype(np.float32),
            res["o_mag"], res["o_ori"], res["o_tmag"],
            res["o_thin"], res["o_tthin"])
